# revision 26
# baseline (speedup 1.0000x reference)
"""Trainium2 Bass kernel for the capsule-routing layer (nn_Caps_Layer).

Computation (per batch b of x [B, S, D], W [D, 25]):
  u_hat = (x_b @ W).reshape(S, 5, 5)           # [S, n, k], col = n*5+k
  b0 = 0;  for 4 routing iters:
    c = softmax_n(b)                            # over the 5 capsules
    v[n,k] = sum_s c[n,s] u_hat[s,n,k]
    out = v / sqrt(sum_k v^2 + 1e-7)
    b[n,s] = sum_k out[n,k] u_hat[s,n,k]
Returns out [B, 5, 5].

Sharding: pure data-parallel over batch across 8 NeuronCores (16 batches
each); W replicated; no collectives.

Per-core pipeline:
  phase 1 (per batch): DMA x rows -> SBUF natural [128s x (4, 768)];
    PE-transpose 128x128 blocks -> xT; matmul W[dblk].T @ xT -> u_hatT
    [25, 512] in PSUM; PE-transpose back -> u_hat natural
    [128 s_lo, (s_hi n k)] gathered per group into UH.
  phase 2 (per group of batches): dynamic routing with
    [128, G*100]-shaped elementwise/reduce ops, partition sums via
    ones-matmul on PE, softmax without max-subtraction (|logits| < ~10).
"""

from contextlib import ExitStack

import numpy as np

import concourse.bass as bass
import concourse.tile as tile
from concourse import mybir, masks

F32 = mybir.dt.float32
F32R = mybir.dt.float32r
BF16 = mybir.dt.bfloat16
AX = mybir.AxisListType
OP = mybir.AluOpType
AF = mybir.ActivationFunctionType

N_CORES = 8
B_FULL, S, D = 128, 512, 768
NCAP, KDIM = 5, 5
NK = NCAP * KDIM  # 25
ROUTINGS = 4
T_EPS = 1e-7

ND = D // 128   # 6 d-blocks
NSB = S // 128  # 4 s-blocks (= s_hi)


def emit(ctx, tc, out, x, w, b_loc=16, group=8, mm_dt=F32, t_dt=F32):
    """Emit the per-core kernel IR.

    out: [1, b_loc*25] f32; x: [b_loc*512, 768] f32; w: [768, 25] f32.
    mm_dt: dtype of the main-matmul inputs (F32 or F32R).
    t_dt: dtype of the routing c*u_hat product feeding partition-sum mms.
    """
    nc = tc.nc
    groups = list(group) if isinstance(group, (list, tuple)) else \
        [group] * (b_loc // group)
    assert sum(groups) == b_loc

    const_pool = ctx.enter_context(tc.tile_pool(name="const", bufs=1))
    xnat_pool = ctx.enter_context(tc.tile_pool(name="xnat", bufs=3))
    ptr_pool = ctx.enter_context(tc.tile_pool(name="ptr", bufs=3, space="PSUM"))
    xt_pool = ctx.enter_context(tc.tile_pool(name="xt", bufs=2))
    pu_pool = ctx.enter_context(tc.tile_pool(name="pu", bufs=2, space="PSUM"))
    uhT_pool = ctx.enter_context(tc.tile_pool(name="uhT", bufs=2))
    uh_pool = ctx.enter_context(tc.tile_pool(name="uh", bufs=2))
    rt_pool = ctx.enter_context(tc.tile_pool(name="rt", bufs=2))
    pv_pool = ctx.enter_context(tc.tile_pool(name="pv", bufs=1, space="PSUM"))

    # --- constants ---
    ident = const_pool.tile([128, 128], F32)
    masks.make_identity(nc, ident[:])
    ident_m = const_pool.tile([128, 128], mm_dt)
    nc.scalar.copy(ident_m[:], ident[:])
    w_raw = const_pool.tile([128, ND * NK], F32)
    # DRAM [768, 25] -> [128, (dblk, nk)]
    nc.sync.dma_start(
        w_raw[:].rearrange("p (nb k) -> p nb k", nb=ND),
        w.rearrange("(nb p) k -> p nb k", p=128),
    )
    w_sb = const_pool.tile([128, ND * NK], mm_dt)
    nc.scalar.copy(w_sb[:], w_raw[:])

    ones_col = const_pool.tile([128, 1], t_dt)
    nc.gpsimd.memset(ones_col[:], 1.0)
    ones_col_f = const_pool.tile([128, 1], F32)
    nc.gpsimd.memset(ones_col_f[:], 1.0)
    ones_row = const_pool.tile([1, 128], F32)
    nc.gpsimd.memset(ones_row[:], 1.0)
    cs_row = const_pool.tile([1, 128], F32)
    nc.gpsimd.memset(cs_row[:], 1.0 / NCAP)

    def warm_pe():
        """Tiny REGULAR matmul: transpose-mode PE activity is invisible to
        the HAM clock gate, so phase 1 otherwise runs at 1.2 GHz. ~110 ns
        of real matmul every few us keeps K=8/8 (2.4 GHz)."""
        wps = pv_pool.tile([1, 64], F32, tag="warm")
        nc.tensor.matmul(wps[:], ones_col_f[:], ident[:, 0:64], start=True, stop=True)
    eps1 = const_pool.tile([1, 1], F32)
    nc.gpsimd.memset(eps1[:], T_EPS)

    b_off = 0
    for g, G in enumerate(groups):
        uh = uh_pool.tile([128, G * NSB * NK], F32, tag="uh")  # [128,(b,s_hi,n,k)]
        for bi in range(G):
            b = b_off + bi
            # --- load x rows for batch b: [512, 768] -> [128, (sblk, d)] ---
            x_nat = xnat_pool.tile([128, NSB * D], mm_dt)
            nc.sync.dma_start(
                x_nat[:].rearrange("p (sb d) -> p sb d", sb=NSB),
                x[b * S:(b + 1) * S, :].rearrange("(sb p) d -> p sb d", p=128),
            )
            # --- transpose to xT [128 d_lo, (dblk, s)]; db-major so each
            # d-block's matmul can issue as soon as its own copy lands ---
            xT = xt_pool.tile([128, ND * S], mm_dt)
            xT3 = xT[:].rearrange("p (db s) -> p db s", db=ND)
            pu = pu_pool.tile([NK, S], F32)
            for db in range(ND):
                ptr = ptr_pool.tile([128, S], mm_dt)
                for sb_i in range(NSB):
                    nc.tensor.transpose(
                        ptr[:, sb_i * 128:(sb_i + 1) * 128],
                        x_nat[:, sb_i * D + db * 128:sb_i * D + (db + 1) * 128],
                        ident_m[:],
                    )
                # copy psum -> sbuf (rounds to f32r when mm_dt is f32r);
                # spread across ACT and DVE to unblock the scalar engine
                if db in (1, 4):
                    nc.vector.tensor_copy(xT3[:, db], ptr[:])
                else:
                    nc.scalar.copy(xT3[:, db], ptr[:])
                nc.tensor.matmul(
                    pu[:],
                    w_sb[:, db * NK:(db + 1) * NK],
                    xT3[:, db],
                    start=(db == 0),
                    stop=(db == ND - 1),
                )
                if db in (2, 5):
                    warm_pe()
            uhT = uhT_pool.tile([NK, S], F32)
            nc.vector.tensor_copy(uhT[:], pu[:])
            # --- transpose back: u_hat natural [128 s_lo, (s_hi, n, k)] ---
            pnat = ptr_pool.tile([128, NSB * NK], F32, tag="ptr")
            for sh in range(NSB):
                nc.tensor.transpose(
                    pnat[:, sh * NK:(sh + 1) * NK],
                    uhT[:, sh * 128:(sh + 1) * 128],
                    ident[0:NK, 0:NK],
                )
            nc.vector.tensor_copy(
                uh[:, bi * NSB * NK:(bi + 1) * NSB * NK], pnat[:]
            )

        # ---------------- routing for this group ----------------
        # Critical-chain-minimized form. Per iter i:
        #   c   = softmax_n(b)                    (skipped on i=0: c = 1/5)
        #   v   = sum_s c*u_hat                   (PE partition-sum, raw v)
        #   rnrm = (cs^2*|v|^2 + eps)^-1/2        (side branch, 1-partition)
        #   b'  = (sum_k v*u_hat) * (cs*rnrm)     (cs folded into the rnrm
        #                                          broadcast matmul's ones)
        # where cs = 1/5 on iter 0 (from the constant softmax), else 1.
        # Final outputs (iter 3) = v * rnrm on partition 0 only.
        uh_ap = uh[:].rearrange("p (b sh n k) -> p b sh n k", b=G, sh=NSB, n=NCAP)
        blog = rt_pool.tile([128, G * NSB * NCAP], F32, tag="blog")
        for it in range(ROUTINGS):
            cs = 1.0 / NCAP if it == 0 else 1.0
            if it == 0:
                t_ap = uh_ap
            else:
                expb = rt_pool.tile([128, G * NSB * NCAP], F32, tag="expb")
                nc.scalar.activation(expb[:], blog[:], AF.Exp)
                den = rt_pool.tile([128, G * NSB], F32, tag="den")
                nc.vector.reduce_sum(
                    den[:],
                    expb[:].rearrange("p (bs n) -> p bs n", n=NCAP),
                    axis=AX.X,
                )
                rden = rt_pool.tile([128, G * NSB], F32, tag="rden")
                nc.vector.reciprocal(rden[:], den[:])
                c = rt_pool.tile([128, G * NSB * NCAP], F32, tag="c")
                nc.vector.tensor_tensor(
                    c[:].rearrange("p (b sh n) -> p b sh n", b=G, sh=NSB),
                    expb[:].rearrange("p (b sh n) -> p b sh n", b=G, sh=NSB),
                    rden[:].rearrange("p (b sh) -> p b sh", b=G)
                    .unsqueeze(3)
                    .broadcast_to((128, G, NSB, NCAP)),
                    op=OP.mult,
                )
                c_b = (
                    c[:]
                    .rearrange("p (b sh n) -> p b sh n", b=G, sh=NSB)
                    .unsqueeze(4)
                    .broadcast_to((128, G, NSB, NCAP, KDIM))
                )
                t = rt_pool.tile([128, G * NSB * NK], t_dt, tag="t")
                t_ap = t[:].rearrange(
                    "p (b sh n k) -> p b sh n k", b=G, sh=NSB, n=NCAP
                )
                nc.vector.tensor_tensor(t_ap, uh_ap, c_b, op=OP.mult)
            # ---- v[n,k] = sum_s t: partition sum via ones matmul ----
            pv = pv_pool.tile([1, G * NK], F32, tag="pv")
            for sh in range(NSB):
                nc.tensor.matmul(
                    pv[:],
                    ones_col[:],
                    t_ap[:, :, sh, :, :],
                    start=(sh == 0),
                    stop=(sh == NSB - 1),
                )
            warm_pe()
            v_sb = rt_pool.tile([1, G * NK], F32, tag="v_sb")
            nc.vector.tensor_copy(v_sb[:], pv[:])
            # ---- side branch: rnrm = (cs^2*|v|^2 + eps)^-1/2 on part 0 ----
            sq = rt_pool.tile([1, G * NK], F32, tag="sq")
            nc.vector.tensor_tensor(sq[:], v_sb[:], v_sb[:], op=OP.mult)
            s2 = rt_pool.tile([1, G * NCAP], F32, tag="s2")
            nc.vector.reduce_sum(
                s2[:], sq[:].rearrange("p (bn k) -> p bn k", k=KDIM), axis=AX.X
            )
            nrm = rt_pool.tile([1, G * NCAP], F32, tag="nrm")
            nc.scalar.activation(
                nrm[:], s2[:], AF.Sqrt, bias=eps1[:], scale=cs * cs
            )
            rnrm = rt_pool.tile([1, G * NCAP], F32, tag="rnrm")
            nc.vector.reciprocal(rnrm[:], nrm[:])
            if it < ROUTINGS - 1:
                # ---- main chain: w = sum_k v*u_hat via pvb broadcast ----
                pvb = pv_pool.tile([128, G * NK], F32, tag="pvb")
                nc.tensor.matmul(pvb[:], ones_row[:], v_sb[:], start=True, stop=True)
                tmp = rt_pool.tile([128, G * NSB * NK], F32, tag="tmp")
                tmp_ap = tmp[:].rearrange(
                    "p (b sh n k) -> p b sh n k", b=G, sh=NSB, n=NCAP
                )
                nc.vector.tensor_tensor(
                    tmp_ap,
                    uh_ap,
                    pvb[:]
                    .rearrange("p (b n k) -> p b n k", b=G, n=NCAP)
                    .unsqueeze(2)
                    .broadcast_to((128, G, NSB, NCAP, KDIM)),
                    op=OP.mult,
                )
                w_t = rt_pool.tile([128, G * NSB * NCAP], F32, tag="w_t")
                nc.vector.reduce_sum(
                    w_t[:],
                    tmp[:].rearrange("p (bsn k) -> p bsn k", k=KDIM),
                    axis=AX.X,
                )
                # broadcast cs*rnrm to all partitions (cs via the ones value)
                prn = pv_pool.tile([128, G * NCAP], F32, tag="pvb")
                nc.tensor.matmul(
                    prn[:],
                    cs_row[:] if it == 0 else ones_row[:],
                    rnrm[:],
                    start=True,
                    stop=True,
                )
                blog = rt_pool.tile([128, G * NSB * NCAP], F32, tag="blog")
                nc.vector.tensor_tensor(
                    blog[:].rearrange("p (b sh n) -> p b sh n", b=G, sh=NSB),
                    w_t[:].rearrange("p (b sh n) -> p b sh n", b=G, sh=NSB),
                    prn[:]
                    .rearrange("p (b n) -> p b n", b=G)
                    .unsqueeze(2)
                    .broadcast_to((128, G, NSB, NCAP)),
                    op=OP.mult,
                )
            else:
                # ---- final outputs on partition 0 (cs == 1 here) ----
                outputs_sb = rt_pool.tile([1, G * NK], F32, tag="outs")
                nc.vector.tensor_tensor(
                    outputs_sb[:].rearrange("p (b n k) -> p b n k", b=G, n=NCAP),
                    v_sb[:].rearrange("p (b n k) -> p b n k", b=G, n=NCAP),
                    rnrm[:]
                    .rearrange("p (b n) -> p b n", b=G)
                    .unsqueeze(3)
                    .broadcast_to((1, G, NCAP, KDIM)),
                    op=OP.mult,
                )
                nc.sync.dma_start(
                    out[0:1, b_off * NK:(b_off + G) * NK],
                    outputs_sb[0:1, :],
                )
        b_off += G


def legalize_waits(nc):
    """This toolchain's walrus codegen accepts at most ONE sync wait per
    instruction ("Too many sync wait commands" otherwise) — and PE Matmult
    appears to take none safely. Hoist excess waits onto wait-only
    EventSemaphore instructions inserted just before, on the same engine
    (same pattern walrus already accepts for Tile's engine barriers)."""
    n = 0
    for fn in nc.m.functions:
        for blk in fn.blocks:
            new = []
            for inst in blk.instructions:
                si = inst.sync_info
                if si is not None and len(si.on_wait) > 0:
                    waits = list(si.on_wait)
                    keep = 0 if type(inst).__name__ == "InstMatmult" else 1
                    if len(waits) > keep:
                        for wt in waits[: len(waits) - keep]:
                            ev = mybir.InstEventSemaphore(
                                name=f"I-waitfix-{nc.next_id()}"
                            )
                            ev.engine = inst.engine
                            ev.sync_info = mybir.SyncInfo(on_wait=[wt], on_update=[])
                            new.append(ev)
                            n += 1
                        si.on_wait = waits[len(waits) - keep:]
                new.append(inst)
            blk.instructions = new
    return n


def build_caps_kernel(b_loc=16, group=8, mm_dt=F32, t_dt=F32):
    nc = bass.Bass(trn_type="TRN2", debug=False, target_bir_lowering=False)
    x = nc.dram_tensor("x", [b_loc * S, D], mm_dt, kind="ExternalInput").ap()
    w = nc.dram_tensor("w", [D, NK], F32, kind="ExternalInput").ap()
    out = nc.dram_tensor("out", [1, b_loc * NK], F32, kind="ExternalOutput").ap()
    with tile.TileContext(nc) as tc:
        with ExitStack() as ctx:
            emit(ctx, tc, out, x, w, b_loc=b_loc, group=group, mm_dt=mm_dt, t_dt=t_dt)
    legalize_waits(nc)
    return nc


_KERNEL_CFG = dict(group=[8, 4, 4], mm_dt=F32R, t_dt=F32)


def kernel(x: np.ndarray, W: np.ndarray) -> np.ndarray:
    from concourse.bass_utils import run_bass_kernel_spmd

    B, S_, D_ = x.shape
    assert (B, S_, D_) == (B_FULL, S, D)
    b_loc = B // N_CORES
    nc = build_caps_kernel(b_loc=b_loc, **_KERNEL_CFG)
    in_maps = [
        {
            "x": np.ascontiguousarray(
                x[i * b_loc:(i + 1) * b_loc].reshape(b_loc * S, D)
            ),
            "w": np.ascontiguousarray(W),
        }
        for i in range(N_CORES)
    ]
    res = run_bass_kernel_spmd(nc, in_maps, core_ids=list(range(N_CORES)))
    outs = [res.results[i]["out"].reshape(b_loc, NCAP, KDIM) for i in range(N_CORES)]
    return np.concatenate(outs, axis=0).astype(np.float32)


# revision 27
# speedup vs baseline: 1.0678x; 1.0678x over previous
"""Trainium2 Bass kernel for the capsule-routing layer (nn_Caps_Layer).

Computation (per batch b of x [B, S, D], W [D, 25]):
  u_hat = (x_b @ W).reshape(S, 5, 5)           # [S, n, k], col = n*5+k
  b0 = 0;  for 4 routing iters:
    c = softmax_n(b)                            # over the 5 capsules
    v[n,k] = sum_s c[n,s] u_hat[s,n,k]
    out = v / sqrt(sum_k v^2 + 1e-7)
    b[n,s] = sum_k out[n,k] u_hat[s,n,k]
Returns out [B, 5, 5].

Sharding: pure data-parallel over batch across 8 NeuronCores (16 batches
each); W replicated; no collectives.

Per-core pipeline:
  phase 1 (per batch): DMA x rows -> SBUF natural [128s x (4, 768)];
    PE-transpose 128x128 blocks -> xT; matmul W[dblk].T @ xT -> u_hatT
    [25, 512] in PSUM; PE-transpose back -> u_hat natural
    [128 s_lo, (s_hi n k)] gathered per group into UH.
  phase 2 (per group of batches): dynamic routing with
    [128, G*100]-shaped elementwise/reduce ops, partition sums via
    ones-matmul on PE, softmax without max-subtraction (|logits| < ~10).
"""

from contextlib import ExitStack

import numpy as np

import concourse.bass as bass
import concourse.tile as tile
from concourse import mybir, masks

F32 = mybir.dt.float32
F32R = mybir.dt.float32r
BF16 = mybir.dt.bfloat16
AX = mybir.AxisListType
OP = mybir.AluOpType
AF = mybir.ActivationFunctionType

N_CORES = 8
B_FULL, S, D = 128, 512, 768
NCAP, KDIM = 5, 5
NK = NCAP * KDIM  # 25
ROUTINGS = 4
T_EPS = 1e-7

ND = D // 128   # 6 d-blocks
NSB = S // 128  # 4 s-blocks (= s_hi)


def emit(ctx, tc, out, x, w, b_loc=16, group=8, mm_dt=F32, t_dt=F32):
    """Emit the per-core kernel IR.

    out: [1, b_loc*25] f32; x: [b_loc*512, 768] f32; w: [768, 25] f32.
    mm_dt: dtype of the main-matmul inputs (F32 or F32R).
    t_dt: dtype of the routing c*u_hat product feeding partition-sum mms.
    """
    nc = tc.nc
    groups = list(group) if isinstance(group, (list, tuple)) else \
        [group] * (b_loc // group)
    assert sum(groups) == b_loc

    const_pool = ctx.enter_context(tc.tile_pool(name="const", bufs=1))
    xnat_pool = ctx.enter_context(tc.tile_pool(name="xnat", bufs=3))
    ptr_pool = ctx.enter_context(tc.tile_pool(name="ptr", bufs=3, space="PSUM"))
    xt_pool = ctx.enter_context(tc.tile_pool(name="xt", bufs=2))
    pu_pool = ctx.enter_context(tc.tile_pool(name="pu", bufs=2, space="PSUM"))
    uhT_pool = ctx.enter_context(tc.tile_pool(name="uhT", bufs=2))
    uh_pool = ctx.enter_context(tc.tile_pool(name="uh", bufs=3))
    rt_pool = ctx.enter_context(tc.tile_pool(name="rt", bufs=2))
    pv_pool = ctx.enter_context(tc.tile_pool(name="pv", bufs=1, space="PSUM"))

    # --- constants ---
    ident = const_pool.tile([128, 128], F32)
    masks.make_identity(nc, ident[:])
    ident_m = const_pool.tile([128, 128], mm_dt)
    nc.scalar.copy(ident_m[:], ident[:])
    w_raw = const_pool.tile([128, ND * NK], F32)
    # DRAM [768, 25] -> [128, (dblk, nk)]
    nc.sync.dma_start(
        w_raw[:].rearrange("p (nb k) -> p nb k", nb=ND),
        w.rearrange("(nb p) k -> p nb k", p=128),
    )
    w_sb = const_pool.tile([128, ND * NK], mm_dt)
    nc.scalar.copy(w_sb[:], w_raw[:])

    ones_col = const_pool.tile([128, 1], t_dt)
    nc.gpsimd.memset(ones_col[:], 1.0)
    ones_col_f = const_pool.tile([128, 1], F32)
    nc.gpsimd.memset(ones_col_f[:], 1.0)
    ones_row = const_pool.tile([1, 128], F32)
    nc.gpsimd.memset(ones_row[:], 1.0)
    cs_row = const_pool.tile([1, 128], F32)
    nc.gpsimd.memset(cs_row[:], 1.0 / NCAP)

    def warm_pe():
        """Tiny REGULAR matmul: transpose-mode PE activity is invisible to
        the HAM clock gate, so phase 1 otherwise runs at 1.2 GHz. ~110 ns
        of real matmul every few us keeps K=8/8 (2.4 GHz)."""
        wps = pv_pool.tile([1, 128], F32, tag="warm")
        nc.tensor.matmul(wps[:], ones_col_f[:], ident[:], start=True, stop=True)
    eps1 = const_pool.tile([1, 1], F32)
    nc.gpsimd.memset(eps1[:], T_EPS)

    # HAM warm-up: ~5us of back-to-back REGULAR matmuls (transpose-mode
    # activity never flips the clock gate to 8/8). Overlaps the first DMA.
    wps = pv_pool.tile([1, 128], F32, tag="warm")
    for _ in range(24):
        nc.tensor.matmul(wps[:], ones_col_f[:], ident[:], start=True, stop=True)

    b_off = 0
    for g, G in enumerate(groups):
        uh = uh_pool.tile([128, G * NSB * NK], F32, tag="uh")  # [128,(b,s_hi,n,k)]
        for bi in range(G):
            b = b_off + bi
            # --- load x rows for batch b: [512, 768] -> [128, (sblk, d)] ---
            x_nat = xnat_pool.tile([128, NSB * D], mm_dt)
            nc.sync.dma_start(
                x_nat[:].rearrange("p (sb d) -> p sb d", sb=NSB),
                x[b * S:(b + 1) * S, :].rearrange("(sb p) d -> p sb d", p=128),
            )
            # --- transpose to xT [128 d_lo, (dblk, s)]; db-major so each
            # d-block's matmul can issue as soon as its own copy lands ---
            xT = xt_pool.tile([128, ND * S], mm_dt)
            xT3 = xT[:].rearrange("p (db s) -> p db s", db=ND)
            pu = pu_pool.tile([NK, S], F32)
            for db in range(ND):
                ptr = ptr_pool.tile([128, S], mm_dt)
                for sb_i in range(NSB):
                    nc.tensor.transpose(
                        ptr[:, sb_i * 128:(sb_i + 1) * 128],
                        x_nat[:, sb_i * D + db * 128:sb_i * D + (db + 1) * 128],
                        ident_m[:],
                    )
                # copy psum -> sbuf (rounds to f32r when mm_dt is f32r);
                # spread across ACT and DVE to unblock the scalar engine
                if db in (1, 4):
                    nc.vector.tensor_copy(xT3[:, db], ptr[:])
                else:
                    nc.scalar.copy(xT3[:, db], ptr[:])
                nc.tensor.matmul(
                    pu[:],
                    w_sb[:, db * NK:(db + 1) * NK],
                    xT3[:, db],
                    start=(db == 0),
                    stop=(db == ND - 1),
                )
                if db in (2, 5):
                    warm_pe()
            uhT = uhT_pool.tile([NK, S], F32)
            nc.vector.tensor_copy(uhT[:], pu[:])
            # --- transpose back: u_hat natural [128 s_lo, (s_hi, n, k)] ---
            pnat = ptr_pool.tile([128, NSB * NK], F32, tag="ptr")
            for sh in range(NSB):
                nc.tensor.transpose(
                    pnat[:, sh * NK:(sh + 1) * NK],
                    uhT[:, sh * 128:(sh + 1) * 128],
                    ident[0:NK, 0:NK],
                )
            nc.vector.tensor_copy(
                uh[:, bi * NSB * NK:(bi + 1) * NSB * NK], pnat[:]
            )

        # ---------------- routing for this group ----------------
        # Critical-chain-minimized form. Per iter i:
        #   c   = softmax_n(b)                    (skipped on i=0: c = 1/5)
        #   v   = sum_s c*u_hat                   (PE partition-sum, raw v)
        #   rnrm = (cs^2*|v|^2 + eps)^-1/2        (side branch, 1-partition)
        #   b'  = (sum_k v*u_hat) * (cs*rnrm)     (cs folded into the rnrm
        #                                          broadcast matmul's ones)
        # where cs = 1/5 on iter 0 (from the constant softmax), else 1.
        # Final outputs (iter 3) = v * rnrm on partition 0 only.
        uh_ap = uh[:].rearrange("p (b sh n k) -> p b sh n k", b=G, sh=NSB, n=NCAP)
        blog = rt_pool.tile([128, G * NSB * NCAP], F32, tag="blog")
        for it in range(ROUTINGS):
            cs = 1.0 / NCAP if it == 0 else 1.0
            if it == 0:
                t_ap = uh_ap
            else:
                expb = rt_pool.tile([128, G * NSB * NCAP], F32, tag="expb")
                nc.scalar.activation(expb[:], blog[:], AF.Exp)
                den = rt_pool.tile([128, G * NSB], F32, tag="den")
                nc.vector.reduce_sum(
                    den[:],
                    expb[:].rearrange("p (bs n) -> p bs n", n=NCAP),
                    axis=AX.X,
                )
                rden = rt_pool.tile([128, G * NSB], F32, tag="rden")
                nc.vector.reciprocal(rden[:], den[:])
                c = rt_pool.tile([128, G * NSB * NCAP], F32, tag="c")
                nc.vector.tensor_tensor(
                    c[:].rearrange("p (b sh n) -> p b sh n", b=G, sh=NSB),
                    expb[:].rearrange("p (b sh n) -> p b sh n", b=G, sh=NSB),
                    rden[:].rearrange("p (b sh) -> p b sh", b=G)
                    .unsqueeze(3)
                    .broadcast_to((128, G, NSB, NCAP)),
                    op=OP.mult,
                )
                c_b = (
                    c[:]
                    .rearrange("p (b sh n) -> p b sh n", b=G, sh=NSB)
                    .unsqueeze(4)
                    .broadcast_to((128, G, NSB, NCAP, KDIM))
                )
                t = rt_pool.tile([128, G * NSB * NK], t_dt, tag="t")
                t_ap = t[:].rearrange(
                    "p (b sh n k) -> p b sh n k", b=G, sh=NSB, n=NCAP
                )
                nc.vector.tensor_tensor(t_ap, uh_ap, c_b, op=OP.mult)
            # ---- v[n,k] = sum_s t: partition sum via ones matmul ----
            pv = pv_pool.tile([1, G * NK], F32, tag="pv")
            for sh in range(NSB):
                nc.tensor.matmul(
                    pv[:],
                    ones_col[:],
                    t_ap[:, :, sh, :, :],
                    start=(sh == 0),
                    stop=(sh == NSB - 1),
                )
            warm_pe()
            v_sb = rt_pool.tile([1, G * NK], F32, tag="v_sb")
            nc.vector.tensor_copy(v_sb[:], pv[:])
            # ---- side branch: rnrm = (cs^2*|v|^2 + eps)^-1/2 on part 0 ----
            sq = rt_pool.tile([1, G * NK], F32, tag="sq")
            nc.vector.tensor_tensor(sq[:], v_sb[:], v_sb[:], op=OP.mult)
            s2 = rt_pool.tile([1, G * NCAP], F32, tag="s2")
            nc.vector.reduce_sum(
                s2[:], sq[:].rearrange("p (bn k) -> p bn k", k=KDIM), axis=AX.X
            )
            nrm = rt_pool.tile([1, G * NCAP], F32, tag="nrm")
            nc.scalar.activation(
                nrm[:], s2[:], AF.Sqrt, bias=eps1[:], scale=cs * cs
            )
            rnrm = rt_pool.tile([1, G * NCAP], F32, tag="rnrm")
            nc.vector.reciprocal(rnrm[:], nrm[:])
            if it < ROUTINGS - 1:
                # ---- main chain: w = sum_k v*u_hat via pvb broadcast ----
                pvb = pv_pool.tile([128, G * NK], F32, tag="pvb")
                nc.tensor.matmul(pvb[:], ones_row[:], v_sb[:], start=True, stop=True)
                tmp = rt_pool.tile([128, G * NSB * NK], F32, tag="tmp")
                tmp_ap = tmp[:].rearrange(
                    "p (b sh n k) -> p b sh n k", b=G, sh=NSB, n=NCAP
                )
                nc.vector.tensor_tensor(
                    tmp_ap,
                    uh_ap,
                    pvb[:]
                    .rearrange("p (b n k) -> p b n k", b=G, n=NCAP)
                    .unsqueeze(2)
                    .broadcast_to((128, G, NSB, NCAP, KDIM)),
                    op=OP.mult,
                )
                w_t = rt_pool.tile([128, G * NSB * NCAP], F32, tag="w_t")
                nc.vector.reduce_sum(
                    w_t[:],
                    tmp[:].rearrange("p (bsn k) -> p bsn k", k=KDIM),
                    axis=AX.X,
                )
                # broadcast cs*rnrm to all partitions (cs via the ones value)
                prn = pv_pool.tile([128, G * NCAP], F32, tag="pvb")
                nc.tensor.matmul(
                    prn[:],
                    cs_row[:] if it == 0 else ones_row[:],
                    rnrm[:],
                    start=True,
                    stop=True,
                )
                blog = rt_pool.tile([128, G * NSB * NCAP], F32, tag="blog")
                nc.vector.tensor_tensor(
                    blog[:].rearrange("p (b sh n) -> p b sh n", b=G, sh=NSB),
                    w_t[:].rearrange("p (b sh n) -> p b sh n", b=G, sh=NSB),
                    prn[:]
                    .rearrange("p (b n) -> p b n", b=G)
                    .unsqueeze(2)
                    .broadcast_to((128, G, NSB, NCAP)),
                    op=OP.mult,
                )
            else:
                # ---- final outputs on partition 0 (cs == 1 here) ----
                outputs_sb = rt_pool.tile([1, G * NK], F32, tag="outs")
                nc.vector.tensor_tensor(
                    outputs_sb[:].rearrange("p (b n k) -> p b n k", b=G, n=NCAP),
                    v_sb[:].rearrange("p (b n k) -> p b n k", b=G, n=NCAP),
                    rnrm[:]
                    .rearrange("p (b n) -> p b n", b=G)
                    .unsqueeze(3)
                    .broadcast_to((1, G, NCAP, KDIM)),
                    op=OP.mult,
                )
                nc.sync.dma_start(
                    out[0:1, b_off * NK:(b_off + G) * NK],
                    outputs_sb[0:1, :],
                )
        b_off += G


def legalize_waits(nc):
    """This toolchain's walrus codegen accepts at most ONE sync wait per
    instruction ("Too many sync wait commands" otherwise) — and PE Matmult
    appears to take none safely. Hoist excess waits onto wait-only
    EventSemaphore instructions inserted just before, on the same engine
    (same pattern walrus already accepts for Tile's engine barriers)."""
    n = 0
    for fn in nc.m.functions:
        for blk in fn.blocks:
            new = []
            for inst in blk.instructions:
                si = inst.sync_info
                if si is not None and len(si.on_wait) > 0:
                    waits = list(si.on_wait)
                    keep = 0 if type(inst).__name__ == "InstMatmult" else 1
                    if len(waits) > keep:
                        for wt in waits[: len(waits) - keep]:
                            ev = mybir.InstEventSemaphore(
                                name=f"I-waitfix-{nc.next_id()}"
                            )
                            ev.engine = inst.engine
                            ev.sync_info = mybir.SyncInfo(on_wait=[wt], on_update=[])
                            new.append(ev)
                            n += 1
                        si.on_wait = waits[len(waits) - keep:]
                new.append(inst)
            blk.instructions = new
    return n


def build_caps_kernel(b_loc=16, group=8, mm_dt=F32, t_dt=F32):
    nc = bass.Bass(trn_type="TRN2", debug=False, target_bir_lowering=False)
    x = nc.dram_tensor("x", [b_loc * S, D], mm_dt, kind="ExternalInput").ap()
    w = nc.dram_tensor("w", [D, NK], F32, kind="ExternalInput").ap()
    out = nc.dram_tensor("out", [1, b_loc * NK], F32, kind="ExternalOutput").ap()
    with tile.TileContext(nc) as tc:
        with ExitStack() as ctx:
            emit(ctx, tc, out, x, w, b_loc=b_loc, group=group, mm_dt=mm_dt, t_dt=t_dt)
    legalize_waits(nc)
    return nc


_KERNEL_CFG = dict(group=8, mm_dt=F32R, t_dt=F32)


def kernel(x: np.ndarray, W: np.ndarray) -> np.ndarray:
    from concourse.bass_utils import run_bass_kernel_spmd

    B, S_, D_ = x.shape
    assert (B, S_, D_) == (B_FULL, S, D)
    b_loc = B // N_CORES
    nc = build_caps_kernel(b_loc=b_loc, **_KERNEL_CFG)
    in_maps = [
        {
            "x": np.ascontiguousarray(
                x[i * b_loc:(i + 1) * b_loc].reshape(b_loc * S, D)
            ),
            "w": np.ascontiguousarray(W),
        }
        for i in range(N_CORES)
    ]
    res = run_bass_kernel_spmd(nc, in_maps, core_ids=list(range(N_CORES)))
    outs = [res.results[i]["out"].reshape(b_loc, NCAP, KDIM) for i in range(N_CORES)]
    return np.concatenate(outs, axis=0).astype(np.float32)
